# revision 23
# baseline (speedup 1.0000x reference)
"""Trainium2 Bass kernel for nn_Attention_53687091200195.

Reference computation (per batch b):
    Q = relu(x @ Wq + bq); K = relu(x @ Wk + bk); V = relu(x @ Wv + bv)
    S = Q @ K^T / sqrt(64); P = softmax(S, axis=-1); out = P @ V

Shapes: x [16, 2048, 64] f32, W* [64, 128] f32, b* [128] f32 -> out [16, 2048, 128].

Sharding: data-parallel over batch. 8 cores x 2 batches each; weights replicated.

Per-core design (SPMD, identical program):
  - Token-permuted layout: the x XBAR-transpose packs even-j token tiles at
    partitions 0..63 and odd-j at 64..127, so the internal tile index is
    m~ = parity*8 + beta (real j = 2*beta + parity) and internal token
    n~ = m~*128 + p maps to real token p*16 + j.  Attention is
    permutation-equivariant; the store AP undoes the permutation.
  - Mixed precision against the 2e-2 gate (measured end-to-end rel err 8.4e-3):
    x/Q/K/weights in bf16, exp output and V in fp8e4, all accumulation fp32.
  - All layout transposes ride the idle DMA queues via the XBAR
    (dma_start_transpose): x -> xT, vT -> v tiles, and the output epilogue.
    The PE does no transposes except the 8 tiny 1-row den transposes/sweep
    (through the pden ring, off the exp-critical pst ring).
  - Projections contract over c=64 per parity half (weights duplicated into
    both partition halves); the bias is added per-partition inside the relu
    (ACT bias operand / DVE tensor_scalar add+max).
  - Attention as one continuous software pipeline over 4 sweeps
    (batch, chunk of 1024 queries): per key tile m: S^T = K_m @ Q^T (bf16),
    E = exp(S^T/8) on ACT -> fp8; PV and the denominator run as fp8
    DoubleRow matmuls over PAIRS of key tiles (contraction 256), halving
    their PE cost; lagged so the PE never waits on the ACT exp stream.
    Cross-sweep: the next sweep's QK/exp start while the previous sweep's
    den tail and epilogue drain, keeping the exp stream gapless (the ACT
    engine at ~1 elem/lane/cycle is the roofline: 64 exps of [128,1024]).
  - Epilogue per sweep: acc -> bf16 outu (DVE), XBAR-transpose to [p, jt, d],
    denominator -> SBUF -> 8 tiny PE transposes -> reciprocal, then a fused
    normalize+fp32-convert on DVE and a contiguous store DMA.
"""

import numpy as np

import concourse.bass as bass
import concourse.mybir as mybir
import concourse.tile as tile
from concourse import bacc
from concourse.bass_utils import run_bass_kernel_spmd

N_CORES = 8
B_PER_CORE = 2
N_TOK = 2048
C_IN = 64
D = 128
P = 128
N_TILES = N_TOK // P          # 16
N_CHUNK = 1024
N_CHUNKS = N_TOK // N_CHUNK   # 2
JT = N_CHUNK // P             # 8
N_SWEEPS = B_PER_CORE * N_CHUNKS  # 4
SCALE = 1.0 / 8.0             # 1/sqrt(64)

F32 = mybir.dt.float32
BF16 = mybir.dt.bfloat16
FP8 = mybir.dt.float8e4
DR = mybir.MatmulPerfMode.DoubleRow
RELU = mybir.ActivationFunctionType.Relu
EXP = mybir.ActivationFunctionType.Exp


def build_program():
    nc = bacc.Bacc("TRN2", target_bir_lowering=False, debug=False,
                   num_devices=N_CORES)

    x = nc.dram_tensor("x", [B_PER_CORE, N_TOK, C_IN], F32, kind="ExternalInput").ap()
    wq = nc.dram_tensor("Wq", [C_IN, D], F32, kind="ExternalInput").ap()
    bq = nc.dram_tensor("bq", [D], F32, kind="ExternalInput").ap()
    wk = nc.dram_tensor("Wk", [C_IN, D], F32, kind="ExternalInput").ap()
    bk = nc.dram_tensor("bk", [D], F32, kind="ExternalInput").ap()
    wv = nc.dram_tensor("Wv", [C_IN, D], F32, kind="ExternalInput").ap()
    bv = nc.dram_tensor("bv", [D], F32, kind="ExternalInput").ap()
    out = nc.dram_tensor("out", [B_PER_CORE, N_TOK, D], F32, kind="ExternalOutput").ap()

    with tile.TileContext(nc) as tc:
        kernel_body(tc, out, x, (wq, bq), (wk, bk), (wv, bv))

    nc.compile()
    return nc


def kernel_body(tc, out, x, qw, kw, vw):
    nc = tc.nc
    from collections import defaultdict
    from contextlib import ExitStack
    ctx = ExitStack()
    with ctx:
        consts = ctx.enter_context(tc.tile_pool(name="consts", bufs=1))
        perb = ctx.enter_context(tc.tile_pool(name="perb", bufs=2))
        epool = ctx.enter_context(tc.tile_pool(name="epool", bufs=2))
        ep = ctx.enter_context(tc.tile_pool(name="ep", bufs=2))
        pst = ctx.enter_context(tc.tile_pool(name="pst", bufs=2, space="PSUM"))
        pacc = ctx.enter_context(tc.tile_pool(name="pacc", bufs=1, space="PSUM"))
        pden = ctx.enter_context(tc.tile_pool(name="pden", bufs=1, space="PSUM"))

        # --- constants ---
        one1 = consts.tile([1, 1], F32)
        nc.vector.memset(one1[:], 1.0)
        # preload the ACT exp table during the head (off the exp stream)
        dummy = consts.tile([1, 1], F32)
        nc.scalar.activation(out=dummy[:], in_=one1[:], func=EXP, scale=SCALE)
        # den stationary: 32 identical columns of ones (walrus rejects
        # DoubleRow ldweights with a 1-wide stationary; cost is row-bound)
        ones2 = consts.tile([P, 2, 32], FP8)
        nc.vector.memset(ones2[:], 1.0)

        # x per batch, token-permuted: x_nat[bb][p, j, c] = x[bb, p*16+j, c].
        x_nat = [consts.tile([P, N_TILES, C_IN], F32, name=f"x_nat_{bb}",
                             tag=f"x_nat_{bb}") for bb in range(B_PER_CORE)]
        H = N_TILES // 2
        for jh in range(2):
            for bb in range(B_PER_CORE):
                eng = nc.sync if bb == 0 else nc.scalar
                eng.dma_start(
                    out=x_nat[bb][:, jh * H:(jh + 1) * H, :],
                    in_=bass.AP(
                        tensor=x.tensor,
                        offset=bb * N_TOK * C_IN + jh * H * C_IN,
                        ap=[[N_TILES * C_IN, P], [C_IN, H], [1, C_IN]],
                    ),
                )
        wfs = {}
        for name, (w, b) in (("q", qw), ("k", kw), ("v", vw)):
            wf = consts.tile([C_IN, D], F32, name=f"wf_{name}", tag=f"wf_{name}")
            nc.scalar.dma_start(out=wf[:], in_=w[:])
            wfs[name] = (wf, b)

        # x -> bf16 (DVE), per jh-half so the XBAR can start after half the DMA
        x_bf = [consts.tile([P, N_TILES, C_IN], BF16, name=f"x_bf_{bb}",
                            tag=f"x_bf_{bb}") for bb in range(B_PER_CORE)]
        for bb in range(B_PER_CORE):
            for jh in range(2):
                nc.vector.tensor_copy(out=x_bf[bb][:, jh * H:(jh + 1) * H, :],
                                      in_=x_nat[bb][:, jh * H:(jh + 1) * H, :])

        # Weights duplicated into both partition halves (the xT layout below
        # packs even-j tiles at partitions 0..63 and odd-j at 64..127); the
        # bias is applied per-partition in the relu instead of bias-folding.
        # The [1,D]->[D,1] bias flip rides an idle DMA queue, off the PE.
        w_sb = {}
        b_col = {}
        for name in ("q", "k", "v"):
            wf, b = wfs[name]
            w2 = consts.tile([2 * C_IN, D], BF16, name=f"w2_{name}", tag=f"w2_{name}")
            nc.vector.tensor_copy(out=w2[0:C_IN, :], in_=wf[:])
            nc.vector.tensor_copy(out=w2[C_IN:2 * C_IN, :], in_=wf[:])
            bc = consts.tile([D, 1], F32, name=f"bc_{name}", tag=f"bc_{name}")
            nc.sync.dma_start(
                out=bc[:],
                in_=bass.AP(tensor=b.tensor, offset=0, ap=[[1, D], [1, 1]]))
            w_sb[name] = w2
            b_col[name] = bc

        # xT via XBAR into the hardware's natural 128-partition layout:
        # xb[bb][alpha, beta, p] = x_bf[bb][p, j, c] with
        #   j = 2*beta + (alpha >= 64), c = alpha % 64.
        # Internal tile index m~ = parity*8 + beta <-> real j = 2*beta+parity,
        # internal token n~ = m~*128 + p <-> real token p*16 + j.  Chunks are
        # therefore the two j-parities; the store AP undoes the permutation.
        xb = [perb.tile([P, JT, P], BF16, name=f"xb_{bb}",
                        tag=f"xb_{bb}", bufs=1) for bb in range(B_PER_CORE)]
        for bb in range(B_PER_CORE):
            nc.sync.dma_start_transpose(out=xb[bb][:], in_=x_bf[bb][:])

        qTs = [perb.tile([D, N_TOK], BF16, name=f"qT_{bb}", tag=f"qT_{bb}",
                         bufs=1) for bb in range(B_PER_CORE)]
        kTs = [perb.tile([D, N_TOK], BF16, name=f"kT_{bb}", tag=f"kT_{bb}",
                         bufs=1) for bb in range(B_PER_CORE)]
        vTs = [perb.tile([D, N_TOK], BF16, name=f"vT_{bb}", tag=f"vT_{bb}",
                         bufs=1) for bb in range(B_PER_CORE)]
        v_bf = [perb.tile([P, N_TILES, D], BF16, name=f"v_bf_{bb}",
                          tag=f"v_bf_{bb}", bufs=1) for bb in range(B_PER_CORE)]
        v_sbs = [perb.tile([P, N_TILES, D], FP8, name=f"v_sb_{bb}",
                           tag=f"v_sb_{bb}", bufs=1) for bb in range(B_PER_CORE)]

        def proj_half(name, bb, par, dst, relu_eng, pool=None):
            """dst[:, par*1024:(par+1)*1024] = relu(x @ W + b) for the 1024
            tokens of j-parity `par` (internal tiles par*8 .. par*8+7).

            `pool` picks the PSUM pool: deferred prologue units run through
            pacc/pden (idle until PV/den start) so they never stall the
            pst ring that feeds the exp stream."""
            pool = pool or pst
            tag = {id(pst): "st", id(pacc): "acc", id(pden): "den"}[id(pool)]
            ps = pool.tile([P, N_CHUNK], F32, tag=tag,
                           name=f"pj_{name}_{bb}_{par}")
            lhsT = w_sb[name][par * C_IN:(par + 1) * C_IN, :]
            for h in range(2):
                nc.tensor.matmul(
                    ps[:, h * 512:(h + 1) * 512], lhsT,
                    xb[bb][par * C_IN:(par + 1) * C_IN, h * 4:(h + 1) * 4, :],
                    start=True, stop=True)
            dslc = dst[:, par * N_CHUNK:(par + 1) * N_CHUNK]
            if relu_eng is nc.scalar:
                nc.scalar.activation(out=dslc, in_=ps[:], func=RELU,
                                     scale=1.0, bias=b_col[name][:])
            else:
                nc.vector.tensor_scalar(
                    out=dslc, in0=ps[:], scalar1=b_col[name][:], scalar2=0.0,
                    op0=mybir.AluOpType.add, op1=mybir.AluOpType.max)

        def v_half(bb, par, pool=None):
            proj_half("v", bb, par, vTs[bb], nc.vector, pool=pool)
            nc.sync.dma_start_transpose(
                out=v_bf[bb][:, par * JT:(par + 1) * JT, :],
                in_=vTs[bb][:, par * N_CHUNK:(par + 1) * N_CHUNK])
            # fp8 convert split so the first PV pairs are ready early
            for lo, nt in ((0, 2), (2, 2), (4, 4)):
                j0 = par * JT + lo
                nc.vector.tensor_copy(
                    out=v_sbs[bb][:, j0:j0 + nt, :],
                    in_=v_bf[bb][:, j0:j0 + nt, :])

        # ---- head: minimum work for sweep 0 (b0 chunk 0) to start ----
        proj_half("q", 0, 0, qTs[0], nc.scalar)
        proj_half("k", 0, 0, kTs[0], nc.scalar)
        v_half(0, 0)

        # ---- deferred prologue + 4 attention sweeps as one pipeline ----
        pending = defaultdict(list)

        def at(g, fn):
            pending[g].append(fn)

        # Deferred prologue: the first two units go through pacc/pden (idle
        # until PV/den start at steps 5/7); the rest ride the pst ring with
        # >=3-step spacing so each relu drains before the ring wraps.
        at(1, lambda: proj_half("k", 0, 1, kTs[0], nc.vector, pool=pden))
        at(2, lambda: proj_half("q", 0, 1, qTs[0], nc.vector, pool=pacc))
        at(3, lambda: v_half(0, 1))
        at(6, lambda: proj_half("k", 1, 0, kTs[1], nc.vector))
        at(6, lambda: proj_half("k", 1, 1, kTs[1], nc.vector))
        at(10, lambda: proj_half("q", 1, 0, qTs[1], nc.vector))
        at(10, lambda: v_half(1, 0))
        at(14, lambda: v_half(1, 1))
        at(14, lambda: proj_half("q", 1, 1, qTs[1], nc.vector))

        for s in range(N_SWEEPS):
            b, c = s // 2, s % 2
            n0 = c * N_CHUNK
            S = {}
            base = 16 * s

            def qk_exp(m, s=s, b=b, n0=n0, S=S):
                if m == 0:
                    S["e"] = epool.tile([P, N_TILES, N_CHUNK], FP8, tag="e",
                                        name=f"e_{s}")
                st = pst.tile([P, N_CHUNK], F32, tag="st", name=f"st_{s}_{m}")
                for h in range(2):
                    nc.tensor.matmul(
                        st[:, h * 512:(h + 1) * 512],
                        kTs[b][:, m * P:(m + 1) * P],
                        qTs[b][:, n0 + h * 512:n0 + (h + 1) * 512],
                        start=True, stop=True)
                nc.scalar.activation(out=S["e"][:, m, :], in_=st[:],
                                     func=EXP, scale=SCALE)

            def pv(p, s=s, b=b, S=S):
                if p == 0:
                    S["acc"] = pacc.tile([P, N_CHUNK], F32, tag="acc",
                                         name=f"acc_{s}")
                for h in range(2):
                    nc.tensor.matmul(
                        S["acc"][:, h * 512:(h + 1) * 512],
                        v_sbs[b][:, 2 * p:2 * p + 2, :],
                        S["e"][:, 2 * p:2 * p + 2, h * 512:(h + 1) * 512],
                        start=(p == 0), stop=(p == 7), perf_mode=DR)

            def dn(q, s=s, S=S):
                if q == 0:
                    S["den"] = pden.tile([32, N_CHUNK], F32, tag="den",
                                         name=f"den_{s}")
                for h in range(2):
                    nc.tensor.matmul(
                        S["den"][:, h * 512:(h + 1) * 512],
                        ones2[:],
                        S["e"][:, 2 * q:2 * q + 2, h * 512:(h + 1) * 512],
                        start=(q == 0), stop=(q == 7), perf_mode=DR)

            def outu_copy(s=s, S=S):
                S["outu"] = ep.tile([P, N_CHUNK], BF16, tag="outu",
                                    name=f"outu_{s}")
                nc.vector.tensor_copy(out=S["outu"][:], in_=S["acc"][:])

            def xbar_out(s=s, S=S):
                S["osbT"] = ep.tile([P, JT, D], BF16, tag="osbT",
                                    name=f"osbT_{s}")
                for half in range(2):
                    nc.sync.dma_start_transpose(
                        out=S["osbT"][:, half * 4:(half + 1) * 4, :],
                        in_=S["outu"][:, half * 512:(half + 1) * 512])

            def den_copy(s=s, S=S):
                S["den_sb"] = ep.tile([1, N_CHUNK], F32, tag="den_sb",
                                      name=f"den_sb_{s}")
                nc.vector.tensor_copy(out=S["den_sb"][:], in_=S["den"][0:1, :])

            def den_tr(s=s, S=S):
                den_t = pden.tile([P, JT], F32, tag="den", name=f"den_t_{s}")
                for jt in range(JT):
                    nc.tensor.transpose(den_t[:, jt:jt + 1],
                                        S["den_sb"][:, jt * P:(jt + 1) * P],
                                        one1[:])
                S["recip"] = ep.tile([P, JT], F32, tag="recip",
                                     name=f"recip_{s}")
                nc.vector.reciprocal(out=S["recip"][:], in_=den_t[:])

            def norm_store(half, s=s, b=b, c=c, S=S):
                if half == 0:
                    S["osb"] = ep.tile([P, JT, D], F32, tag="osb",
                                       name=f"osb_{s}")
                for jt in range(half * 4, (half + 1) * 4):
                    nc.vector.tensor_scalar(
                        out=S["osb"][:, jt, :], in0=S["osbT"][:, jt, :],
                        scalar1=S["recip"][:, jt:jt + 1], scalar2=None,
                        op0=mybir.AluOpType.mult)
                # internal (p, jt) of chunk c -> real token p*16 + 2*jt + c
                nc.sync.dma_start(
                    out=bass.AP(
                        tensor=out.tensor,
                        offset=(b * N_TOK + c + 2 * half * 4) * D,
                        ap=[[N_TILES * D, P], [2 * D, 4], [1, D]],
                    ),
                    in_=S["osb"][:, half * 4:(half + 1) * 4, :],
                )

            for m in range(N_TILES):
                at(base + m, lambda m=m, f=qk_exp: f(m))
            for p in range(8):
                at(base + (2 * p + 5 if p < 6 else 11 + p), lambda p=p, f=pv: f(p))
            for q in range(8):
                at(base + (2 * q + 7 if q < 6 else 12 + q), lambda q=q, f=dn: f(q))
            at(base + 19, outu_copy)
            at(base + 20, xbar_out)
            at(base + 20, den_copy)
            at(base + 22, den_tr)
            at(base + 23, lambda f=norm_store: f(0))
            at(base + 24, lambda f=norm_store: f(1))

        for g in range(16 * (N_SWEEPS - 1) + 25):
            for fn in pending.get(g, ()):
                fn()


_NC_CACHE = None


def _get_program():
    global _NC_CACHE
    if _NC_CACHE is None:
        _NC_CACHE = build_program()
    return _NC_CACHE


def kernel(x, Wq, bq, Wk, bk, Wv, bv, _trace=False):
    x = np.ascontiguousarray(np.asarray(x, dtype=np.float32))
    full_b = x.shape[0]
    assert full_b == N_CORES * B_PER_CORE, x.shape
    nc = _get_program()
    common = {
        "Wq": np.ascontiguousarray(np.asarray(Wq, np.float32)),
        "bq": np.ascontiguousarray(np.asarray(bq, np.float32)),
        "Wk": np.ascontiguousarray(np.asarray(Wk, np.float32)),
        "bk": np.ascontiguousarray(np.asarray(bk, np.float32)),
        "Wv": np.ascontiguousarray(np.asarray(Wv, np.float32)),
        "bv": np.ascontiguousarray(np.asarray(bv, np.float32)),
    }
    in_maps = [
        {"x": x[c * B_PER_CORE:(c + 1) * B_PER_CORE], **common}
        for c in range(N_CORES)
    ]
    res = run_bass_kernel_spmd(nc, in_maps, list(range(N_CORES)), trace=_trace)
    outs = np.concatenate([res.results[c]["out"] for c in range(N_CORES)], axis=0)
    if _trace:
        kernel.last_exec_time_ns = res.exec_time_ns
    return outs


# revision 24
# speedup vs baseline: 1.0242x; 1.0242x over previous
"""Trainium2 Bass kernel for nn_Attention_53687091200195.

Reference computation (per batch b):
    Q = relu(x @ Wq + bq); K = relu(x @ Wk + bk); V = relu(x @ Wv + bv)
    S = Q @ K^T / sqrt(64); P = softmax(S, axis=-1); out = P @ V

Shapes: x [16, 2048, 64] f32, W* [64, 128] f32, b* [128] f32 -> out [16, 2048, 128].

Sharding: data-parallel over batch. 8 cores x 2 batches each; weights replicated.

Per-core design (SPMD, identical program):
  - Token-permuted layout: the x XBAR-transpose packs even-j token tiles at
    partitions 0..63 and odd-j at 64..127, so the internal tile index is
    m~ = parity*8 + beta (real j = 2*beta + parity) and internal token
    n~ = m~*128 + p maps to real token p*16 + j.  Attention is
    permutation-equivariant; the store AP undoes the permutation.
  - Mixed precision against the 2e-2 gate (measured end-to-end rel err 8.4e-3):
    x/Q/K/weights in bf16, exp output and V in fp8e4, all accumulation fp32.
  - All layout transposes ride the idle DMA queues via the XBAR
    (dma_start_transpose): x -> xT, vT -> v tiles, and the output epilogue.
    The PE does no transposes except the 8 tiny 1-row den transposes/sweep
    (through the pden ring, off the exp-critical pst ring).
  - Projections contract over c=64 per parity half (weights duplicated into
    both partition halves); the bias is added per-partition inside the relu
    (ACT bias operand / DVE tensor_scalar add+max).
  - Attention as one continuous software pipeline over 4 sweeps
    (batch, chunk of 1024 queries): per key tile m: S^T = K_m @ Q^T (bf16),
    E = exp(S^T/8) on ACT -> fp8; PV and the denominator run as fp8
    DoubleRow matmuls over PAIRS of key tiles (contraction 256), halving
    their PE cost; lagged so the PE never waits on the ACT exp stream.
    Cross-sweep: the next sweep's QK/exp start while the previous sweep's
    den tail and epilogue drain, keeping the exp stream gapless (the ACT
    engine at ~1 elem/lane/cycle is the roofline: 64 exps of [128,1024]).
  - Epilogue per sweep: acc -> bf16 outu (DVE), XBAR-transpose to [p, jt, d],
    denominator -> SBUF -> 8 tiny PE transposes -> reciprocal, then a fused
    normalize+fp32-convert on DVE and a contiguous store DMA.
"""

import numpy as np

import concourse.bass as bass
import concourse.mybir as mybir
import concourse.tile as tile
from concourse import bacc
from concourse.bass_utils import run_bass_kernel_spmd

N_CORES = 8
B_PER_CORE = 2
N_TOK = 2048
C_IN = 64
D = 128
P = 128
N_TILES = N_TOK // P          # 16
N_CHUNK = 1024
N_CHUNKS = N_TOK // N_CHUNK   # 2
JT = N_CHUNK // P             # 8
N_SWEEPS = B_PER_CORE * N_CHUNKS  # 4
SCALE = 1.0 / 8.0             # 1/sqrt(64)

F32 = mybir.dt.float32
BF16 = mybir.dt.bfloat16
FP8 = mybir.dt.float8e4
DR = mybir.MatmulPerfMode.DoubleRow
RELU = mybir.ActivationFunctionType.Relu
EXP = mybir.ActivationFunctionType.Exp


def build_program():
    nc = bacc.Bacc("TRN2", target_bir_lowering=False, debug=False,
                   num_devices=N_CORES)

    x = nc.dram_tensor("x", [B_PER_CORE, N_TOK, C_IN], F32, kind="ExternalInput").ap()
    wq = nc.dram_tensor("Wq", [C_IN, D], F32, kind="ExternalInput").ap()
    bq = nc.dram_tensor("bq", [D], F32, kind="ExternalInput").ap()
    wk = nc.dram_tensor("Wk", [C_IN, D], F32, kind="ExternalInput").ap()
    bk = nc.dram_tensor("bk", [D], F32, kind="ExternalInput").ap()
    wv = nc.dram_tensor("Wv", [C_IN, D], F32, kind="ExternalInput").ap()
    bv = nc.dram_tensor("bv", [D], F32, kind="ExternalInput").ap()
    out = nc.dram_tensor("out", [B_PER_CORE, N_TOK, D], F32, kind="ExternalOutput").ap()

    with tile.TileContext(nc) as tc:
        kernel_body(tc, out, x, (wq, bq), (wk, bk), (wv, bv))

    nc.compile()
    return nc


def kernel_body(tc, out, x, qw, kw, vw):
    nc = tc.nc
    from collections import defaultdict
    from contextlib import ExitStack
    ctx = ExitStack()
    with ctx:
        consts = ctx.enter_context(tc.tile_pool(name="consts", bufs=1))
        perb = ctx.enter_context(tc.tile_pool(name="perb", bufs=2))
        epool = ctx.enter_context(tc.tile_pool(name="epool", bufs=2))
        ep = ctx.enter_context(tc.tile_pool(name="ep", bufs=2))
        pst = ctx.enter_context(tc.tile_pool(name="pst", bufs=2, space="PSUM"))
        pacc = ctx.enter_context(tc.tile_pool(name="pacc", bufs=1, space="PSUM"))
        pden = ctx.enter_context(tc.tile_pool(name="pden", bufs=1, space="PSUM"))

        # --- constants ---
        one1 = consts.tile([1, 1], F32)
        nc.vector.memset(one1[:], 1.0)
        # preload the ACT exp table during the head (off the exp stream)
        dummy = consts.tile([1, 1], F32)
        nc.scalar.activation(out=dummy[:], in_=one1[:], func=EXP, scale=SCALE)
        # den stationary: 32 identical columns of ones (walrus rejects
        # DoubleRow ldweights with a 1-wide stationary; cost is row-bound)
        ones2 = consts.tile([P, 2, 32], FP8)
        nc.vector.memset(ones2[:], 1.0)

        # x per batch, token-permuted: x_nat[bb][p, j, c] = x[bb, p*16+j, c].
        x_nat = [consts.tile([P, N_TILES, C_IN], F32, name=f"x_nat_{bb}",
                             tag=f"x_nat_{bb}") for bb in range(B_PER_CORE)]
        H = N_TILES // 2
        for jh in range(2):
            for bb in range(B_PER_CORE):
                eng = nc.sync if bb == 0 else nc.scalar
                eng.dma_start(
                    out=x_nat[bb][:, jh * H:(jh + 1) * H, :],
                    in_=bass.AP(
                        tensor=x.tensor,
                        offset=bb * N_TOK * C_IN + jh * H * C_IN,
                        ap=[[N_TILES * C_IN, P], [C_IN, H], [1, C_IN]],
                    ),
                )
        wfs = {}
        for name, (w, b) in (("q", qw), ("k", kw), ("v", vw)):
            wf = consts.tile([C_IN, D], F32, name=f"wf_{name}", tag=f"wf_{name}")
            nc.scalar.dma_start(out=wf[:], in_=w[:])
            wfs[name] = (wf, b)

        # x -> bf16 (DVE), per jh-half so the XBAR can start after half the DMA
        x_bf = [consts.tile([P, N_TILES, C_IN], BF16, name=f"x_bf_{bb}",
                            tag=f"x_bf_{bb}") for bb in range(B_PER_CORE)]
        for bb in range(B_PER_CORE):
            for jh in range(2):
                nc.vector.tensor_copy(out=x_bf[bb][:, jh * H:(jh + 1) * H, :],
                                      in_=x_nat[bb][:, jh * H:(jh + 1) * H, :])

        # Weights duplicated into both partition halves (the xT layout below
        # packs even-j tiles at partitions 0..63 and odd-j at 64..127); the
        # bias is applied per-partition in the relu instead of bias-folding.
        # The [1,D]->[D,1] bias flip rides an idle DMA queue, off the PE.
        w_sb = {}
        b_col = {}
        for name in ("q", "k", "v"):
            wf, b = wfs[name]
            w2 = consts.tile([2 * C_IN, D], BF16, name=f"w2_{name}", tag=f"w2_{name}")
            nc.vector.tensor_copy(out=w2[0:C_IN, :], in_=wf[:])
            nc.vector.tensor_copy(out=w2[C_IN:2 * C_IN, :], in_=wf[:])
            bc = consts.tile([D, 1], F32, name=f"bc_{name}", tag=f"bc_{name}")
            nc.sync.dma_start(
                out=bc[:],
                in_=bass.AP(tensor=b.tensor, offset=0, ap=[[1, D], [1, 1]]))
            w_sb[name] = w2
            b_col[name] = bc

        # xT via XBAR into the hardware's natural 128-partition layout:
        # xb[bb][alpha, beta, p] = x_bf[bb][p, j, c] with
        #   j = 2*beta + (alpha >= 64), c = alpha % 64.
        # Internal tile index m~ = parity*8 + beta <-> real j = 2*beta+parity,
        # internal token n~ = m~*128 + p <-> real token p*16 + j.  Chunks are
        # therefore the two j-parities; the store AP undoes the permutation.
        xb = [perb.tile([P, JT, P], BF16, name=f"xb_{bb}",
                        tag=f"xb_{bb}", bufs=1) for bb in range(B_PER_CORE)]
        for bb in range(B_PER_CORE):
            nc.sync.dma_start_transpose(out=xb[bb][:], in_=x_bf[bb][:])

        qTs = [perb.tile([D, N_TOK], BF16, name=f"qT_{bb}", tag=f"qT_{bb}",
                         bufs=1) for bb in range(B_PER_CORE)]
        kTs = [perb.tile([D, N_TOK], BF16, name=f"kT_{bb}", tag=f"kT_{bb}",
                         bufs=1) for bb in range(B_PER_CORE)]
        vTs = [perb.tile([D, N_TOK], BF16, name=f"vT_{bb}", tag=f"vT_{bb}",
                         bufs=1) for bb in range(B_PER_CORE)]
        v_bf = [perb.tile([P, N_TILES, D], BF16, name=f"v_bf_{bb}",
                          tag=f"v_bf_{bb}", bufs=1) for bb in range(B_PER_CORE)]
        v_sbs = [perb.tile([P, N_TILES, D], FP8, name=f"v_sb_{bb}",
                           tag=f"v_sb_{bb}", bufs=1) for bb in range(B_PER_CORE)]

        def proj_half(name, bb, par, dst, relu_eng, pool=None):
            """dst[:, par*1024:(par+1)*1024] = relu(x @ W + b) for the 1024
            tokens of j-parity `par` (internal tiles par*8 .. par*8+7).

            `pool` picks the PSUM pool: deferred prologue units run through
            pacc/pden (idle until PV/den start) so they never stall the
            pst ring that feeds the exp stream."""
            pool = pool or pst
            tag = {id(pst): "st", id(pacc): "acc", id(pden): "den"}[id(pool)]
            ps = pool.tile([P, N_CHUNK], F32, tag=tag,
                           name=f"pj_{name}_{bb}_{par}")
            lhsT = w_sb[name][par * C_IN:(par + 1) * C_IN, :]
            for h in range(2):
                nc.tensor.matmul(
                    ps[:, h * 512:(h + 1) * 512], lhsT,
                    xb[bb][par * C_IN:(par + 1) * C_IN, h * 4:(h + 1) * 4, :],
                    start=True, stop=True)
            dslc = dst[:, par * N_CHUNK:(par + 1) * N_CHUNK]
            if relu_eng is nc.scalar:
                nc.scalar.activation(out=dslc, in_=ps[:], func=RELU,
                                     scale=1.0, bias=b_col[name][:])
            else:
                nc.vector.tensor_scalar(
                    out=dslc, in0=ps[:], scalar1=b_col[name][:], scalar2=0.0,
                    op0=mybir.AluOpType.add, op1=mybir.AluOpType.max)

        def v_half(bb, par, pool=None):
            proj_half("v", bb, par, vTs[bb], nc.vector, pool=pool)
            nc.sync.dma_start_transpose(
                out=v_bf[bb][:, par * JT:(par + 1) * JT, :],
                in_=vTs[bb][:, par * N_CHUNK:(par + 1) * N_CHUNK])
            # fp8 convert split so the first PV pairs are ready early
            for lo, nt in ((0, 2), (2, 2), (4, 4)):
                j0 = par * JT + lo
                nc.vector.tensor_copy(
                    out=v_sbs[bb][:, j0:j0 + nt, :],
                    in_=v_bf[bb][:, j0:j0 + nt, :])

        # ---- head: minimum work for sweep 0 (b0 chunk 0) to start ----
        proj_half("q", 0, 0, qTs[0], nc.scalar)
        proj_half("k", 0, 0, kTs[0], nc.scalar)
        v_half(0, 0)

        # ---- deferred prologue + 4 attention sweeps as one pipeline ----
        pending = defaultdict(list)

        def at(g, fn):
            pending[g].append(fn)

        # Deferred prologue: the first two units go through pacc/pden (idle
        # until PV/den start at steps 5/7); the rest ride the pst ring with
        # >=3-step spacing so each relu drains before the ring wraps.
        at(1, lambda: proj_half("k", 0, 1, kTs[0], nc.vector, pool=pden))
        at(2, lambda: proj_half("q", 0, 1, qTs[0], nc.vector, pool=pacc))
        at(3, lambda: v_half(0, 1))
        at(6, lambda: proj_half("k", 1, 0, kTs[1], nc.vector))
        at(9, lambda: proj_half("k", 1, 1, kTs[1], nc.vector))
        at(12, lambda: proj_half("q", 1, 0, qTs[1], nc.vector))
        at(18, lambda: v_half(1, 0))
        at(21, lambda: v_half(1, 1))
        at(24, lambda: proj_half("q", 1, 1, qTs[1], nc.vector))

        for s in range(N_SWEEPS):
            b, c = s // 2, s % 2
            n0 = c * N_CHUNK
            S = {}
            base = 16 * s

            def qk_exp(m, s=s, b=b, n0=n0, S=S):
                if m == 0:
                    S["e"] = epool.tile([P, N_TILES, N_CHUNK], FP8, tag="e",
                                        name=f"e_{s}")
                st = pst.tile([P, N_CHUNK], F32, tag="st", name=f"st_{s}_{m}")
                for h in range(2):
                    nc.tensor.matmul(
                        st[:, h * 512:(h + 1) * 512],
                        kTs[b][:, m * P:(m + 1) * P],
                        qTs[b][:, n0 + h * 512:n0 + (h + 1) * 512],
                        start=True, stop=True)
                nc.scalar.activation(out=S["e"][:, m, :], in_=st[:],
                                     func=EXP, scale=SCALE)

            def pv(p, s=s, b=b, S=S):
                if p == 0:
                    S["acc"] = pacc.tile([P, N_CHUNK], F32, tag="acc",
                                         name=f"acc_{s}")
                for h in range(2):
                    nc.tensor.matmul(
                        S["acc"][:, h * 512:(h + 1) * 512],
                        v_sbs[b][:, 2 * p:2 * p + 2, :],
                        S["e"][:, 2 * p:2 * p + 2, h * 512:(h + 1) * 512],
                        start=(p == 0), stop=(p == 7), perf_mode=DR)

            def dn(q, s=s, S=S):
                if q == 0:
                    S["den"] = pden.tile([32, N_CHUNK], F32, tag="den",
                                         name=f"den_{s}")
                for h in range(2):
                    nc.tensor.matmul(
                        S["den"][:, h * 512:(h + 1) * 512],
                        ones2[:],
                        S["e"][:, 2 * q:2 * q + 2, h * 512:(h + 1) * 512],
                        start=(q == 0), stop=(q == 7), perf_mode=DR)

            def outu_copy(s=s, S=S):
                S["outu"] = ep.tile([P, N_CHUNK], BF16, tag="outu",
                                    name=f"outu_{s}")
                nc.vector.tensor_copy(out=S["outu"][:], in_=S["acc"][:])

            def xbar_out(s=s, S=S):
                S["osbT"] = ep.tile([P, JT, D], BF16, tag="osbT",
                                    name=f"osbT_{s}")
                for half in range(2):
                    nc.sync.dma_start_transpose(
                        out=S["osbT"][:, half * 4:(half + 1) * 4, :],
                        in_=S["outu"][:, half * 512:(half + 1) * 512])

            def den_copy(s=s, S=S):
                S["den_sb"] = ep.tile([1, N_CHUNK], F32, tag="den_sb",
                                      name=f"den_sb_{s}")
                nc.vector.tensor_copy(out=S["den_sb"][:], in_=S["den"][0:1, :])

            def den_tr(s=s, S=S):
                den_t = pden.tile([P, JT], F32, tag="den", name=f"den_t_{s}")
                for jt in range(JT):
                    nc.tensor.transpose(den_t[:, jt:jt + 1],
                                        S["den_sb"][:, jt * P:(jt + 1) * P],
                                        one1[:])
                S["recip"] = ep.tile([P, JT], F32, tag="recip",
                                     name=f"recip_{s}")
                nc.vector.reciprocal(out=S["recip"][:], in_=den_t[:])

            def norm_store(half, s=s, b=b, c=c, S=S):
                if half == 0:
                    S["osb"] = ep.tile([P, JT, D], F32, tag="osb",
                                       name=f"osb_{s}")
                for jt in range(half * 4, (half + 1) * 4):
                    nc.vector.tensor_scalar(
                        out=S["osb"][:, jt, :], in0=S["osbT"][:, jt, :],
                        scalar1=S["recip"][:, jt:jt + 1], scalar2=None,
                        op0=mybir.AluOpType.mult)
                # internal (p, jt) of chunk c -> real token p*16 + 2*jt + c
                nc.sync.dma_start(
                    out=bass.AP(
                        tensor=out.tensor,
                        offset=(b * N_TOK + c + 2 * half * 4) * D,
                        ap=[[N_TILES * D, P], [2 * D, 4], [1, D]],
                    ),
                    in_=S["osb"][:, half * 4:(half + 1) * 4, :],
                )

            for m in range(N_TILES):
                at(base + m, lambda m=m, f=qk_exp: f(m))
            for p in range(8):
                at(base + (2 * p + 5 if p < 6 else 11 + p), lambda p=p, f=pv: f(p))
            for q in range(8):
                at(base + (2 * q + 7 if q < 6 else 12 + q), lambda q=q, f=dn: f(q))
            at(base + 19, outu_copy)
            at(base + 20, xbar_out)
            at(base + 20, den_copy)
            at(base + 22, den_tr)
            at(base + 23, lambda f=norm_store: f(0))
            at(base + 24, lambda f=norm_store: f(1))

        for g in range(16 * (N_SWEEPS - 1) + 25):
            for fn in pending.get(g, ()):
                fn()


_NC_CACHE = None


def _get_program():
    global _NC_CACHE
    if _NC_CACHE is None:
        _NC_CACHE = build_program()
    return _NC_CACHE


def kernel(x, Wq, bq, Wk, bk, Wv, bv, _trace=False):
    x = np.ascontiguousarray(np.asarray(x, dtype=np.float32))
    full_b = x.shape[0]
    assert full_b == N_CORES * B_PER_CORE, x.shape
    nc = _get_program()
    common = {
        "Wq": np.ascontiguousarray(np.asarray(Wq, np.float32)),
        "bq": np.ascontiguousarray(np.asarray(bq, np.float32)),
        "Wk": np.ascontiguousarray(np.asarray(Wk, np.float32)),
        "bk": np.ascontiguousarray(np.asarray(bk, np.float32)),
        "Wv": np.ascontiguousarray(np.asarray(Wv, np.float32)),
        "bv": np.ascontiguousarray(np.asarray(bv, np.float32)),
    }
    in_maps = [
        {"x": x[c * B_PER_CORE:(c + 1) * B_PER_CORE], **common}
        for c in range(N_CORES)
    ]
    res = run_bass_kernel_spmd(nc, in_maps, list(range(N_CORES)), trace=_trace)
    outs = np.concatenate([res.results[c]["out"] for c in range(N_CORES)], axis=0)
    if _trace:
        kernel.last_exec_time_ns = res.exec_time_ns
    return outs
